# revision 1
# baseline (speedup 1.0000x reference)
"""Fused linear + cross-entropy loss on 8 Trainium2 NeuronCores.

Strategy: vocab-parallel. The weight [V=50257, d=2048] is padded to
8*6400=51200 rows and sharded over V across the 8 cores. Every core gets
all 4096 tokens (x) and computes, per token:
  - sumexp_m = sum_v exp(logit[t, v])   over its local 6400 vocab columns
  - pickedexp_m = exp(logit[t, label[t]]) if the label is in its shard else 0
No max-subtraction is needed: logits are ~N(0,1) (|logit| < 10), so
exp never overflows fp32. The host merges the tiny per-token stats:
  lse = log(sum_m sumexp_m - n_pad), picked = log(sum_m pickedexp_m),
  loss = mean(lse - picked) + 1e-4*mean(lse^2).
No collectives are used at all.

Per-core device kernel (Tile framework):
  - x^T [2048, 4096] resident in SBUF, K-major ([128, 16, 4096])
  - W^T shard [2048, 6400] streamed in 4 column-groups (3x2048 + 256)
  - logits tile [128 tok, 1024 v] accumulated in PSUM (2 banks) over K;
    each 512-wide bank is its own matmul accumulation group, with the
    K-pair loop OUTER (j_outer): consecutive matmuls share the stationary
    x operand, so LDWEIGHTS (the DoubleRow weight-load penalty) hides
    under two matmuls' streaming instead of one (~10-18% faster on HW)
  - ScalarE: exp(scale*psum) + per-partition sum in ONE activation instr
    per (t,G) tile (accum_out)
  - VectorE: picked via ONE scalar_tensor_tensor instr reading the exp'd
    SBUF tile: (iota == label-col) * exp(logits), accum_out = row-sum
    (reads SBUF, not PSUM, to avoid PSUM same-bank port contention)

dtype_mode:
  - "fp8": float8_e4m3 inputs, W pre-scaled by W_SCALE=64 (so W entries
    ~N(0,1) avoid fp8 subnormals); DoubleRow matmuls (2 K-rows/instr).
    PSUM holds 64*logits; exp uses activation scale=1/64. HW loss rel err
    ~2e-5.
  - "bf16": plain bf16 matmuls, loss rel err ~1.4e-6, ~2x slower.
"""

import numpy as np
import ml_dtypes

import concourse.bass as bass
import concourse.mybir as mybir
import concourse.tile as tile
from concourse import bacc
from concourse.bass_utils import run_bass_kernel_spmd

# ---- problem constants (hardcoded per contract) ----
D = 2048            # in_features (contraction)
V_TOTAL = 50257     # vocab
N_CORES = 8
V_LOC = 6400        # padded per-core vocab shard (8*6400 = 51200)
N_PAD = N_CORES * V_LOC - V_TOTAL  # 943 zero rows, all on core 7
N_TOK = 4096        # 2*2048 tokens
KT = D // 128       # 16 contraction chunks
G_SIZES = [1280] + [1024] * 5          # V column groups, fp8. The odd 256
                                       # columns ride as the first group's 3rd
                                       # (partial) bank so j_outer stationary-
                                       # sharing hides their LDWEIGHTS too
G_SIZES_BF16 = [512] * 12 + [256]      # bf16 tiles are 2x bytes; keep SBUF fit


def g_sizes_for(mode):
    return G_SIZES if mode == "fp8" else G_SIZES_BF16
IGNORE_INDEX = -100
Z_REG = 1e-4
W_SCALE = 64.0      # fp8 mode: W pre-scale (power of 2: exact to undo)
F32 = mybir.dt.float32
BF16 = mybir.dt.bfloat16
FP8 = mybir.dt.float8e4
NP_FP8 = mybir.dt.np(FP8)          # ml_dtypes.float8_e4m3
DTYPE_MODE = "fp8"


def build_nc(n_tok=N_TOK, v_loc=V_LOC, g_sizes=None, repeats=1,
             dtype_mode=DTYPE_MODE, psum_bufs=4, scratch_bufs=4,
             j_outer=True):
    """Build the per-core Bass program (same program on all 8 cores).

    repeats>1 re-runs the whole compute loop (for marginal HW timing only).
    """
    if g_sizes is None:
        g_sizes = g_sizes_for(dtype_mode)
    assert n_tok % 128 == 0 and sum(g_sizes) == v_loc
    t_tiles = n_tok // 128
    ng = len(g_sizes)
    ncols = t_tiles * ng
    gmax = max(g_sizes)
    fp8 = dtype_mode == "fp8"
    in_dt = FP8 if fp8 else BF16
    exp_scale = (1.0 / W_SCALE) if fp8 else 1.0
    banks_per_tile = (gmax + 511) // 512
    psum_bufs = max(1, min(psum_bufs, 8 // banks_per_tile))

    nc = bacc.Bacc(None, target_bir_lowering=False, debug=False)

    xT = nc.dram_tensor("xT", [D, n_tok], in_dt, kind="ExternalInput")
    wT = nc.dram_tensor("wT", [D, v_loc], in_dt, kind="ExternalInput")
    iota = nc.dram_tensor("iota", [128, gmax], F32, kind="ExternalInput")
    lab = nc.dram_tensor("lab", [128, ncols], F32, kind="ExternalInput")
    sumexp = nc.dram_tensor("sumexp", [128, ncols], F32, kind="ExternalOutput")
    picked = nc.dram_tensor("picked", [128, ncols], F32, kind="ExternalOutput")

    xT_r = xT.rearrange("(k p) n -> p k n", p=128)   # [128, KT, n_tok]
    wT_r = wT.rearrange("(k p) v -> p k v", p=128)   # [128, KT, v_loc]

    with tile.TileContext(nc) as tc:
        with (
            tc.tile_pool(name="xpool", bufs=1) as xpool,
            tc.tile_pool(name="wpool", bufs=2) as wpool,
            tc.tile_pool(name="cpool", bufs=1) as cpool,
            tc.tile_pool(name="spool", bufs=scratch_bufs) as spool,
            tc.tile_pool(name="ppool", bufs=psum_bufs, space=bass.MemorySpace.PSUM) as ppool,
        ):
            # group 0's W goes first in the DMA queue: it is small and
            # gates the first matmuls; x streams in behind it
            w0_sb = wpool.tile([128, KT, gmax], in_dt, tag="w")
            nc.sync.dma_start(out=w0_sb[:, :, : g_sizes[0]], in_=wT_r[:, :, : g_sizes[0]])

            # x lives in 8 independent k-pair tiles: Tile tracks deps per
            # tile, so the first matmuls start once pair 0 has landed
            # instead of waiting for the whole 8 MB of x
            x_pairs = []
            for j in range(KT // 2):
                xp = xpool.tile([128, 2, n_tok], in_dt, tag=f"x{j}")
                nc.sync.dma_start(out=xp[:], in_=xT_r[:, 2 * j : 2 * j + 2, :])
                x_pairs.append(xp)
            iota_sb = cpool.tile([128, gmax], F32)
            nc.sync.dma_start(out=iota_sb[:], in_=iota[:])
            lab_sb = cpool.tile([128, ncols], F32)
            nc.sync.dma_start(out=lab_sb[:], in_=lab[:])

            se_acc = cpool.tile([128, ncols], F32)
            pk_acc = cpool.tile([128, ncols], F32)

            for _rep in range(repeats):
                off = 0
                for g, gs in enumerate(g_sizes):
                    if g == 0 and _rep == 0:
                        w_sb = w0_sb
                    else:
                        w_sb = wpool.tile([128, KT, gmax], in_dt, tag="w")
                        nc.sync.dma_start(
                            out=w_sb[:, :, :gs], in_=wT_r[:, :, off : off + gs]
                        )
                    for t in range(t_tiles):
                        ps = ppool.tile([128, gmax], F32, tag="ps")
                        tok = slice(t * 128, (t + 1) * 128)
                        # each 512-wide PSUM bank is its own accumulation group
                        nbank = (gs + 511) // 512
                        if fp8 and j_outer:
                            # share the stationary x-pair across banks:
                            # consecutive matmuls reuse the loaded weights
                            for j in range(KT // 2):
                                for b in range(nbank):
                                    vs = min(512, gs - b * 512)
                                    bank = slice(b * 512, b * 512 + vs)
                                    nc.tensor.matmul(
                                        ps[:, bank],
                                        x_pairs[j][:, :, tok],
                                        w_sb[:, 2 * j : 2 * j + 2, bank],
                                        start=(j == 0),
                                        stop=(j == KT // 2 - 1),
                                        perf_mode=mybir.MatmulPerfMode.DoubleRow,
                                        skip_group_check=True,
                                    )
                        elif fp8:
                            for b in range(nbank):
                                vs = min(512, gs - b * 512)
                                bank = slice(b * 512, b * 512 + vs)
                                for j in range(KT // 2):
                                    nc.tensor.matmul(
                                        ps[:, bank],
                                        x_pairs[j][:, :, tok],
                                        w_sb[:, 2 * j : 2 * j + 2, bank],
                                        start=(j == 0),
                                        stop=(j == KT // 2 - 1),
                                        perf_mode=mybir.MatmulPerfMode.DoubleRow,
                                    )
                        else:
                            for b in range(nbank):
                                vs = min(512, gs - b * 512)
                                bank = slice(b * 512, b * 512 + vs)
                                for k in range(KT):
                                    nc.tensor.matmul(
                                        ps[:, bank],
                                        x_pairs[k // 2][:, k % 2, tok],
                                        w_sb[:, k, bank],
                                        start=(k == 0),
                                        stop=(k == KT - 1),
                                    )
                        col = t * ng + g
                        ex = spool.tile([128, gmax], F32, tag="ex")
                        nc.scalar.activation(
                            ex[:, :gs],
                            ps[:, :gs],
                            mybir.ActivationFunctionType.Exp,
                            scale=exp_scale,
                            accum_out=se_acc[:, col : col + 1],
                        )
                        mk = spool.tile([128, gmax], F32, tag="mk")
                        nc.vector.scalar_tensor_tensor(
                            out=mk[:, :gs],
                            in0=iota_sb[:, :gs],
                            scalar=lab_sb[:, col : col + 1],
                            in1=ex[:, :gs],
                            op0=mybir.AluOpType.is_equal,
                            op1=mybir.AluOpType.mult,
                            accum_out=pk_acc[:, col : col + 1],
                        )
                    off += gs

            nc.sync.dma_start(out=sumexp[:], in_=se_acc[:])
            nc.sync.dma_start(out=picked[:], in_=pk_acc[:])

    nc.compile()
    return nc


def make_in_maps(x, labels, weight, n_tok=N_TOK, v_loc=V_LOC, g_sizes=None,
                 n_cores=N_CORES, dtype_mode=DTYPE_MODE):
    """Host-side prep: transpose/cast inputs, build per-core input maps."""
    if g_sizes is None:
        g_sizes = g_sizes_for(dtype_mode)
    t_tiles = n_tok // 128
    ng = len(g_sizes)
    gmax = max(g_sizes)
    g_offs = np.cumsum([0] + list(g_sizes[:-1])).astype(np.float32)
    fp8 = dtype_mode == "fp8"
    np_dt = NP_FP8 if fp8 else ml_dtypes.bfloat16

    xf = np.ascontiguousarray(x.reshape(n_tok, D).T).astype(np_dt)
    w_eff = weight * np.float32(W_SCALE) if fp8 else weight
    wb = w_eff.astype(np_dt)
    v_total = weight.shape[0]
    wpad = np.zeros((n_cores * v_loc, D), np_dt)
    wpad[:v_total] = wb

    iota_arr = np.ascontiguousarray(
        np.broadcast_to(np.arange(gmax, dtype=np.float32), (128, gmax))
    )
    lab_flat = labels.reshape(-1).astype(np.int64)
    # token (t, p) = t*128 + p lives at [p, t]
    lab_pt = lab_flat.reshape(t_tiles, 128).T.astype(np.float64)  # [128, T]

    in_maps = []
    for m in range(n_cores):
        wT_m = np.ascontiguousarray(wpad[m * v_loc : (m + 1) * v_loc].T)
        ll = lab_pt - m * v_loc  # [128, T]
        lab_m = (ll[:, :, None] - g_offs[None, None, :]).reshape(128, t_tiles * ng)
        in_maps.append(
            {
                "xT": xf,
                "wT": wT_m,
                "iota": iota_arr,
                "lab": np.ascontiguousarray(lab_m.astype(np.float32)),
            }
        )
    return in_maps


def merge_results(results, labels, n_tok=N_TOK, g_sizes=None, n_pad=N_PAD,
                  dtype_mode=DTYPE_MODE):
    """Host-side merge of per-core [128, T*NG] stats into the scalar loss."""
    if g_sizes is None:
        g_sizes = g_sizes_for(dtype_mode)
    t_tiles = n_tok // 128
    ng = len(g_sizes)
    se = np.stack([np.asarray(r["sumexp"], np.float64) for r in results])
    pk = np.stack([np.asarray(r["picked"], np.float64) for r in results])
    # [cores, 128, T, NG] -> per-token [cores, n_tok] (token = t*128+p)
    se_tok = se.reshape(-1, 128, t_tiles, ng).sum(3).transpose(0, 2, 1).reshape(-1, n_tok)
    pk_tok = pk.reshape(-1, 128, t_tiles, ng).sum(3).transpose(0, 2, 1).reshape(-1, n_tok)
    sumexp_tok = se_tok.sum(0) - float(n_pad)  # padding rows give exp(0)=1 each
    pickedexp_tok = pk_tok.sum(0)              # = exp(picked logit)

    lab_flat = labels.reshape(-1).astype(np.int64)
    valid = lab_flat != IGNORE_INDEX
    n_valid = float(valid.sum())
    denom = max(n_valid, 1.0)
    lse = np.log(sumexp_tok)
    picked = np.log(np.where(pickedexp_tok > 0, pickedexp_tok, 1.0))
    nll = lse - picked
    loss = np.where(valid, nll, 0.0).sum() / denom
    if Z_REG > 0.0 and n_valid > 0:
        loss = loss + Z_REG * np.where(valid, lse * lse, 0.0).sum() / denom
    return np.float32(loss)


_CACHE = {}


def kernel(x, labels, weight):
    x = np.asarray(x, dtype=np.float32)
    labels_np = np.asarray(labels)
    weight = np.asarray(weight, dtype=np.float32)

    if "nc" not in _CACHE:
        _CACHE["nc"] = build_nc()
    nc = _CACHE["nc"]

    in_maps = make_in_maps(x, labels_np, weight)
    res = run_bass_kernel_spmd(nc, in_maps, core_ids=list(range(N_CORES)))
    return merge_results(res.results, labels_np)

